# revision 12
# baseline (speedup 1.0000x reference)
"""Two-layer GAT (N=4096, 4 heads, HID=256) on 8 TRN2 NeuronCores.

Sharding: each core owns N/8 = 512 destination rows of every N^2 attention
matrix. Weights are replicated. Per head we compute the local projection
g_shard = h_shard @ W.T on the owning core, then AllGather a packed
[512, 257] payload (g | s_dst) so every core has the full [4096, 257]
g for the attention matmul.

Rank-1 softmax decomposition: because exp is monotone,
  exp(leakyrelu(u)) = max(e^u, e^{0.2u}),  u = s_src_i + s_dst_j
and on each branch e^u = e^{s_i} e^{s_j} is a rank-1 outer product. With
binary gates T1 = [u >= 0] * M and T2 = [u < 0] * M (M = adjacency):
  P = T1 o (a b^T) + T2 o (c d^T),   a=e^{s_src}, b=e^{s_dst},
                                     c=a^0.2,     d=b^0.2
so P @ g = a o (T1 @ (b o g)) + c o (T2 @ (d o g)): the N^2 exp/lrelu/mask
elementwise work of a direct softmax collapses to producing the two binary
matrices. T1/T2 come from one shared tensor_tensor add u' = sbc + Vnan
(Vnan = 0 where adjacent, NaN where not -- NaN fails both >= and <
comparisons, killing masked entries in both gates) plus two 4x-mode
tensor_scalar compares. The i-side scalings fold into the epilogue:
  h' = (U1 + e o U2) / (D1 + e o D2),  e_i = c_i/a_i = e^{-0.8 s_i}.

Layout: attention tiles are [j=source (partition), i=dest (free)] so the
binary gates feed the PE matmul directly as lhsT. U12[ib] is a single
PSUM bank [128, 512] = [T1@(b o g) | T2@(d o g)]; the denominators
accumulate via tiny N=2 matmuls (rhs = [b|d] column pair) that reuse the
already-loaded gate weights and hide under the LDWEIGHTS shadow.

elu is computed +1-shifted (elu(x)+1 = relu(x) + min(exp(x),1)) and the
-1 correction is folded into the next matmul's bias on the host
(outb -= out_w @ 1).

All matmul operands are bf16; accumulation is fp32 in PSUM.
"""

import os

import numpy as np
import ml_dtypes

import concourse.bass as bass
import concourse.tile as tile
from concourse import bacc, mybir
from concourse.bass_utils import run_bass_kernel_spmd

N, IN_DIM, HID, HEADS, OUT_DIM = 4096, 768, 256, 4, 32
ALPHA = 0.2
NCORES = 8
R = N // NCORES          # 512 rows per core
RB = R // 128            # 4 row blocks
FB = IN_DIM // 128       # 6 feature blocks
JB = N // 128            # 32 source chunks
HC = (HID * HEADS) // 128  # 8 concat-feature chunks
GW = HID + 1             # payload width: g (256) | s_dst

F32 = mybir.dt.float32
BF16 = mybir.dt.bfloat16
AF = mybir.ActivationFunctionType
OP = mybir.AluOpType

last_exec_time_ns = None
_nc_cache = None

# engine for the u' = sbc + Vnan pass per source chunk: D = DVE, G = GPSIMD
U_ENG = list("DDGDDGDDGDDGDDGDDGDDGDDGDDGDDGDD")
assert len(U_ENG) == 32


def _build_layer(nc, tc, pools, x_tiles, W_ap, WT_ap, ap_ap, vnan_all, L):
    """One GAT layer. x_tiles: 6 SBUF tiles [128, R] bf16 (features x rows,
    feature-major). Returns 8 SBUF tiles [128, R] bf16 = concat-head
    activations transposed (x_gatT), elu+1 applied."""
    sb = pools["sb"]
    ps_acc = pools["ps_acc"]
    ps_big = pools["ps_big"]
    ps_dens = pools["ps_dens"]
    dram_pay = pools["dram_pay"]
    dram_gat = pools["dram_gat"]
    ones_row = pools["ones_row"]

    groups = [list(range(NCORES))]

    head_state = []
    # ---- Phase A: per-head projection + payload + AllGather ----
    for h in range(HEADS):
        # weights for this head
        W_t = []
        for cc in range(2):
            wt = sb.tile([128, IN_DIM], BF16, name=f"W_L{L}h{h}c{cc}", tag="Wh", bufs=4)
            nc.sync.dma_start(out=wt[:, :], in_=W_ap[h, cc * 128:(cc + 1) * 128, :])
            W_t.append(wt)
        WTaug = []
        for fb in range(FB):
            wta = sb.tile([128, HID + 2], BF16, name=f"WTa_L{L}h{h}f{fb}", tag="WTaug",
                          bufs=2 * FB)
            nc.sync.dma_start(out=wta[:, 0:HID], in_=WT_ap[h, fb * 128:(fb + 1) * 128, :])
            WTaug.append(wta)
        a_t = []
        for cc in range(2):
            at = sb.tile([128, 2], BF16, name=f"a_L{L}h{h}c{cc}", tag="ah", bufs=4)
            nc.sync.dma_start(out=at[:, :], in_=ap_ap[h, cc * 128:(cc + 1) * 128, :])
            a_t.append(at)

        # w_eff[f, 0:2] = W.T @ [a_src | a_dst]  -> [768, 2] in 6 blocks
        weff = []
        for fb in range(FB):
            pw = ps_big.tile([128, 512], F32, name=f"pw_L{L}h{h}f{fb}", tag="ps_big")
            for cc in range(2):
                nc.tensor.matmul(pw[:, 0:2], lhsT=W_t[cc][:, fb * 128:(fb + 1) * 128],
                                 rhs=a_t[cc][:, :], start=(cc == 0), stop=(cc == 1))
            wf = sb.tile([128, 2], BF16, name=f"weff_L{L}h{h}f{fb}", tag="weff",
                         bufs=2 * FB)
            nc.vector.tensor_copy(wf[:, :], pw[:, 0:2])
            # dst half -> column HID of the projection rhs (s_dst per row)
            nc.vector.tensor_copy(WTaug[fb][:, HID:HID + 1], wf[:, 1:2])
            # src half -> column HID+1 (s_src per row, for the epilogue scalars)
            nc.vector.tensor_copy(WTaug[fb][:, HID + 1:HID + 2], wf[:, 0:1])
            weff.append(wf)

        # s_srcT [1, R] = w_eff_src.T @ x  (free-axis layout for the broadcast)
        ps_s = ps_big.tile([128, R], F32, name=f"ps_s_L{L}h{h}", tag="ps_big")
        for fb in range(FB):
            nc.tensor.matmul(ps_s[0:1, :], lhsT=weff[fb][:, 0:1], rhs=x_tiles[fb][:, :],
                             start=(fb == 0), stop=(fb == FB - 1))
        ssrcT = sb.tile([1, R], BF16, name=f"ssrcT_L{L}h{h}", tag="ssrcT", bufs=2)
        nc.vector.tensor_copy(ssrcT[:, :], ps_s[0:1, :])

        # broadcast s_src across partitions: [128, R] bf16 (feeds the gate ops)
        pb = ps_big.tile([128, R], F32, name=f"pb_L{L}h{h}", tag="ps_big")
        nc.tensor.matmul(pb[:, :], lhsT=ones_row[0:1, :], rhs=ssrcT[:, :],
                         start=True, stop=True)
        sbc = sb.tile([128, R], BF16, name=f"sbc_L{L}h{h}", tag="sbc", bufs=HEADS)
        nc.vector.tensor_copy(sbc[:, :], pb[:, :])

        # g_aug = x.T @ WTaug -> [512, 258] (g | s_dst | s_src)
        pay_t = dram_pay.tile([R, GW], BF16, name=f"pay_L{L}h{h}", tag="pay")
        pl = sb.tile([128, RB * GW], BF16, name=f"pl_L{L}h{h}", tag="pl", bufs=2)
        ecol = sb.tile([128, RB], F32, name=f"ecol_L{L}h{h}", tag="ecol", bufs=HEADS)
        for ib in range(RB):
            pg = ps_acc.tile([128, 512], F32, name=f"pg_L{L}h{h}b{ib}", tag="ps_acc")
            for fb in range(FB):
                nc.tensor.matmul(pg[:, 0:HID + 2],
                                 lhsT=x_tiles[fb][:, ib * 128:(ib + 1) * 128],
                                 rhs=WTaug[fb][:, :], start=(fb == 0),
                                 stop=(fb == FB - 1))
            o = ib * GW
            nc.vector.tensor_copy(pl[:, o:o + HID], pg[:, 0:HID])
            nc.vector.tensor_copy(pl[:, o + HID:o + HID + 1], pg[:, HID:HID + 1])
            # e_i = e^{-0.8 s_src_i}: per-partition epilogue scalar
            nc.scalar.activation(ecol[:, ib:ib + 1], pg[:, HID + 1:HID + 2], AF.Exp,
                                 scale=-0.8)
        # one DMA: SBUF [p, (ib, c)] -> DRAM [(ib, p), c]
        nc.sync.dma_start(out=pay_t.rearrange("(ib p) c -> p ib c", p=128),
                          in_=pl.rearrange("p (ib c) -> p ib c", c=GW))

        gat_t = dram_gat.tile([N, GW], BF16, name=f"gat_L{L}h{h}", tag="gat",
                              addr_space="Shared")
        nc.gpsimd.collective_compute(
            "AllGather", OP.bypass, replica_groups=groups,
            ins=[pay_t.opt()], outs=[gat_t.opt()],
        )
        head_state.append((gat_t, sbc, ecol))

    # ---- Phase B: attention per head ----
    xgatT = []
    for hc in range(HC):
        xg = sb.tile([128, R], BF16, name=f"xgatT_L{L}c{hc}", tag="xgatT", bufs=HC)
        xgatT.append(xg)

    for h in range(HEADS):
        gat_t, sbc, ecol = head_state[h]
        # one DMA: DRAM [(jj, p), c] -> SBUF [p, (jj, c)]
        gf = sb.tile([128, JB * GW], BF16, name=f"gf_L{L}h{h}", tag="gf", bufs=2)
        for q in range(4):
            js = slice(q * JB // 4, (q + 1) * JB // 4)
            nc.sync.dma_start(out=gf.rearrange("p (j c) -> p j c", c=GW)[:, js],
                              in_=gat_t.rearrange("(j p) c -> p j c", p=128)[:, js])
        # all 32 s_dst columns (stride GW) as one fp32 strip
        sdst = sb.tile([128, JB], F32, name=f"sdst_L{L}h{h}", tag="sdst", bufs=2)
        for q in range(4):
            js = slice(q * JB // 4, (q + 1) * JB // 4)
            nc.vector.tensor_copy(
                sdst[:, js],
                gf.rearrange("p (j c) -> p j c", c=GW)[:, js, GW - 1])
        # b = e^{s_dst}, d = e^{0.2 s_dst} strips, interleaved [b|d] per jj;
        # bf16 copy feeds the tiny denominator matmuls, fp32 feeds TS scalars
        bd32 = sb.tile([128, 2 * JB], F32, name=f"bd32_L{L}h{h}", tag="bd32", bufs=2)
        bd16 = sb.tile([128, 2 * JB], BF16, name=f"bd16_L{L}h{h}", tag="bd16", bufs=2)
        bdv32 = bd32.rearrange("p (j two) -> p j two", two=2)
        bdv16 = bd16.rearrange("p (j two) -> p j two", two=2)
        nc.scalar.activation(bdv32[:, :, 0], sdst[:, :], AF.Exp)
        nc.scalar.activation(bdv32[:, :, 1], sdst[:, :], AF.Exp, scale=ALPHA)
        nc.vector.tensor_copy(bd16[:, :], bd32[:, :])

        U12 = []
        for ib in range(RB):
            u = ps_acc.tile([128, 2 * HID], F32, name=f"U_L{L}h{h}b{ib}", tag="ps_acc")
            U12.append(u)
        dens = ps_dens.tile([128, 512], F32, name=f"dens_L{L}h{h}", tag="ps_dens")

        for jj in range(JB):
            o = jj * GW
            # u' = sbc + Vnan (NaN where not adjacent -> both gates 0)
            up = sb.tile([128, R], BF16, name=f"up_L{L}h{h}j{jj}", tag="up", bufs=3)
            eng = nc.gpsimd if U_ENG[jj] == "G" else nc.vector
            eng.tensor_tensor(up[:, :], sbc[:, :], vnan_all[:, jj * R:(jj + 1) * R],
                              OP.add)
            # binary gates (4x-mode tensor_scalar compares)
            t1 = sb.tile([128, R], BF16, name=f"t1_L{L}h{h}j{jj}", tag="t1", bufs=3)
            nc.vector.tensor_scalar(t1[:, :], up[:, :], sdst[:, jj:jj + 1], 0.0,
                                    OP.add, OP.is_ge)
            t2 = sb.tile([128, R], BF16, name=f"t2_L{L}h{h}j{jj}", tag="t2", bufs=3)
            nc.vector.tensor_scalar(t2[:, :], up[:, :], sdst[:, jj:jj + 1], 0.0,
                                    OP.add, OP.is_lt)
            # scaled payload [b o g | d o g]
            x12 = sb.tile([128, 2 * HID], BF16, name=f"x12_L{L}h{h}j{jj}", tag="x12",
                          bufs=3)
            nc.vector.tensor_scalar(x12[:, 0:HID], gf[:, o:o + HID],
                                    bd32[:, 2 * jj:2 * jj + 1], None, OP.mult)
            nc.scalar.activation(x12[:, HID:2 * HID], gf[:, o:o + HID], AF.Copy,
                                 scale=bd32[:, 2 * jj + 1:2 * jj + 2])
            # PSUM zero-region semantics: start=True zeroes the whole 2KB bank,
            # so exactly ONE matmul per bank per accumulation round carries
            # start=True (the first in program order); the other groups' first
            # writes land on pending-zero bytes and start fresh. Only the last
            # matmul per bank carries stop=True. The group checker (enabled)
            # validates the emission order.
            first, last = (jj == 0), (jj == JB - 1)
            for ib in range(RB):
                ibs = slice(ib * 128, (ib + 1) * 128)
                nc.tensor.matmul(U12[ib][:, 0:HID], lhsT=t1[:, ibs],
                                 rhs=x12[:, 0:HID], start=first, stop=False)
                nc.tensor.matmul(dens[:, 4 * ib:4 * ib + 2], lhsT=t1[:, ibs],
                                 rhs=bd16[:, 2 * jj:2 * jj + 2],
                                 start=(first and ib == 0), stop=False)
                nc.tensor.matmul(U12[ib][:, HID:2 * HID], lhsT=t2[:, ibs],
                                 rhs=x12[:, HID:2 * HID], start=False,
                                 stop=last)
                nc.tensor.matmul(dens[:, 4 * ib + 2:4 * ib + 4], lhsT=t2[:, ibs],
                                 rhs=bd16[:, 2 * jj:2 * jj + 2], start=False,
                                 stop=(last and ib == RB - 1))

        densS = sb.tile([128, 4 * RB], F32, name=f"densS_L{L}h{h}", tag="densS",
                        bufs=2)
        nc.vector.tensor_copy(densS[:, :], dens[:, 0:4 * RB])
        for ib in range(RB):
            # num = U1 + e o U2 ; den = D1 + e * D2 ; hn = num/den
            tmp = sb.tile([128, HID], F32, name=f"tmp_L{L}h{h}b{ib}", tag="tmp", bufs=2)
            nc.scalar.activation(tmp[:, :], U12[ib][:, HID:2 * HID], AF.Copy,
                                 scale=ecol[:, ib:ib + 1])
            num = sb.tile([128, HID], F32, name=f"num_L{L}h{h}b{ib}", tag="num", bufs=2)
            nc.vector.tensor_tensor(num[:, :], U12[ib][:, 0:HID], tmp[:, :], OP.add)
            dsum = sb.tile([128, 1], F32, name=f"dsum_L{L}h{h}b{ib}", tag="dsum",
                           bufs=2)
            nc.vector.scalar_tensor_tensor(dsum[:, :], densS[:, 4 * ib + 3:4 * ib + 4],
                                           ecol[:, ib:ib + 1],
                                           densS[:, 4 * ib:4 * ib + 1],
                                           OP.mult, OP.add)
            rcp = sb.tile([128, 1], F32, name=f"rcp_L{L}h{h}b{ib}", tag="rcp", bufs=2)
            nc.vector.reciprocal(rcp[:, :], dsum[:, :])
            hn = sb.tile([128, HID], BF16, name=f"hn_L{L}h{h}b{ib}", tag="hn", bufs=2)
            nc.vector.tensor_scalar(hn[:, :], num[:, :], rcp[:, 0:1], None, OP.mult)
            # elu+1 = relu(hn) + min(exp(hn), 1)
            r1 = sb.tile([128, HID], BF16, name=f"r1_L{L}h{h}b{ib}", tag="r1", bufs=2)
            nc.vector.tensor_scalar(r1[:, :], hn[:, :], 0.0, None, OP.max)
            e0 = sb.tile([128, HID], BF16, name=f"e0_L{L}h{h}b{ib}", tag="e0", bufs=2)
            nc.scalar.activation(e0[:, :], hn[:, :], AF.Exp)
            eo = sb.tile([128, HID], BF16, name=f"eo_L{L}h{h}b{ib}", tag="eo", bufs=2)
            nc.vector.scalar_tensor_tensor(eo[:, :], e0[:, :], 1.0, r1[:, :],
                                           OP.min, OP.add)
            for cb in range(2):
                nc.sync.dma_start_transpose(
                    out=xgatT[h * 2 + cb][:, ib * 128:(ib + 1) * 128],
                    in_=eo[:, cb * 128:(cb + 1) * 128])
    return xgatT


def _build_program():
    nc = bacc.Bacc("TRN2", target_bir_lowering=False, debug=False,
                   num_devices=NCORES)

    xT_in = nc.dram_tensor("xT", [IN_DIM, R], BF16, kind="ExternalInput").ap()
    vnan_in = nc.dram_tensor("vnan", [N, R], BF16, kind="ExternalInput").ap()
    W1_in = nc.dram_tensor("W1", [HEADS, HID, IN_DIM], BF16, kind="ExternalInput").ap()
    W1T_in = nc.dram_tensor("W1T", [HEADS, IN_DIM, HID], BF16, kind="ExternalInput").ap()
    a1_in = nc.dram_tensor("a1p", [HEADS, HID, 2], BF16, kind="ExternalInput").ap()
    W2_in = nc.dram_tensor("W2", [HEADS, HID, IN_DIM], BF16, kind="ExternalInput").ap()
    W2T_in = nc.dram_tensor("W2T", [HEADS, IN_DIM, HID], BF16, kind="ExternalInput").ap()
    a2_in = nc.dram_tensor("a2p", [HEADS, HID, 2], BF16, kind="ExternalInput").ap()
    outwT_in = nc.dram_tensor("outwT", [HID * HEADS, IN_DIM], BF16,
                              kind="ExternalInput").ap()
    outb_in = nc.dram_tensor("outb", [IN_DIM, 1], F32, kind="ExternalInput").ap()
    out2wT_in = nc.dram_tensor("out2wT", [HID * HEADS, OUT_DIM], BF16,
                               kind="ExternalInput").ap()
    out2b_in = nc.dram_tensor("out2b", [OUT_DIM, 1], F32, kind="ExternalInput").ap()
    outT = nc.dram_tensor("outT", [OUT_DIM, R], F32, kind="ExternalOutput").ap()

    with tile.TileContext(nc) as tc:
        with tc.tile_pool(name="sb", bufs=1) as sb, \
             tc.tile_pool(name="ps_acc", bufs=RB, space="PSUM") as ps_acc, \
             tc.tile_pool(name="ps_big", bufs=2, space="PSUM") as ps_big, \
             tc.tile_pool(name="ps_dens", bufs=2, space="PSUM") as ps_dens, \
             tc.tile_pool(name="dram_pay", bufs=4, space="DRAM") as dram_pay, \
             tc.tile_pool(name="dram_gat", bufs=3, space="DRAM") as dram_gat:

            pools = dict(sb=sb, ps_acc=ps_acc, ps_big=ps_big, ps_dens=ps_dens,
                         dram_pay=dram_pay, dram_gat=dram_gat)

            groups = [list(range(NCORES))]

            # warm up the collective path (absorbs the one-time barrier) with a
            # tiny gather issued before anything else on the gpsimd queue
            warm = sb.tile([1, 16], BF16, name="warm", tag="warm", bufs=1)
            nc.vector.memset(warm[:, :], 0.0)
            wpay = dram_pay.tile([1, 16], BF16, name="wpay", tag="wpay")
            nc.sync.dma_start(out=wpay[:, :], in_=warm[:, :])
            wgat = dram_gat.tile([NCORES, 16], BF16, name="wgat", tag="wgat",
                                 addr_space="Shared")
            nc.gpsimd.collective_compute(
                "AllGather", OP.bypass, replica_groups=groups,
                ins=[wpay.opt()], outs=[wgat.opt()],
            )

            ones_row = sb.tile([1, 128], BF16, name="ones_row", tag="ones_row", bufs=1)
            nc.vector.memset(ones_row[:, :], 1.0)
            pools["ones_row"] = ones_row

            # resident inputs
            x0 = []
            for fb in range(FB):
                x = sb.tile([128, R], BF16, name=f"x0_{fb}", tag="x0", bufs=FB)
                nc.sync.dma_start(out=x[:, :], in_=xT_in[fb * 128:(fb + 1) * 128, :])
                x0.append(x)
            outw_t = []
            for hc in range(HC):
                w = sb.tile([128, IN_DIM], BF16, name=f"outw{hc}", tag="outw", bufs=HC)
                nc.sync.dma_start(out=w[:, :], in_=outwT_in[hc * 128:(hc + 1) * 128, :])
                outw_t.append(w)
            out2w_t = []
            for hc in range(HC):
                w = sb.tile([128, OUT_DIM], BF16, name=f"out2w{hc}", tag="out2w",
                            bufs=HC)
                nc.sync.dma_start(out=w[:, :], in_=out2wT_in[hc * 128:(hc + 1) * 128, :])
                out2w_t.append(w)
            outb_t = []
            for fb in range(FB):
                b = sb.tile([128, 1], F32, name=f"outb{fb}", tag="outb", bufs=FB)
                nc.sync.dma_start(out=b[:, :], in_=outb_in[fb * 128:(fb + 1) * 128, :])
                outb_t.append(b)
            out2b_t = sb.tile([OUT_DIM, 1], F32, name="out2b", tag="out2b", bufs=1)
            nc.sync.dma_start(out=out2b_t[:, :], in_=out2b_in[:, :])

            # adjacency gate-killer: 0 where adjacent, NaN where not
            vnan_all = sb.tile([128, JB * R], BF16, name="vnan_all", tag="vnan",
                               bufs=1)
            for q in range(4):
                js = slice(q * JB // 4, (q + 1) * JB // 4)
                nc.sync.dma_start(
                    out=vnan_all.rearrange("p (j c) -> p j c", c=R)[:, js],
                    in_=vnan_in.rearrange("(j p) c -> p j c", p=128)[:, js])

            # ---- layer 1 ----
            xg1 = _build_layer(nc, tc, pools, x0, W1_in, W1T_in, a1_in, vnan_all, 1)
            x1 = []
            for fb in range(FB):
                px = ps_big.tile([128, R], F32, name=f"px1_{fb}", tag="ps_big")
                for hc in range(HC):
                    nc.tensor.matmul(px[:, :], lhsT=outw_t[hc][:, fb * 128:(fb + 1) * 128],
                                     rhs=xg1[hc][:, :], start=(hc == 0),
                                     stop=(hc == HC - 1))
                x = sb.tile([128, R], BF16, name=f"x1_{fb}", tag="x1", bufs=FB)
                nc.vector.tensor_scalar(x[:, :], px[:, :], outb_t[fb][:, 0:1], None,
                                        OP.add)
                x1.append(x)

            # ---- layer 2 ----
            xg2 = _build_layer(nc, tc, pools, x1, W2_in, W2T_in, a2_in, vnan_all, 2)
            po = ps_big.tile([OUT_DIM, R], F32, name="po", tag="ps_big")
            for hc in range(HC):
                nc.tensor.matmul(po[:, :], lhsT=out2w_t[hc][:, 0:OUT_DIM],
                                 rhs=xg2[hc][:, :], start=(hc == 0),
                                 stop=(hc == HC - 1))
            ot = sb.tile([OUT_DIM, R], F32, name="ot", tag="ot", bufs=1)
            nc.vector.tensor_scalar(ot[:, :], po[:, :], out2b_t[:, 0:1], None, OP.add)
            nc.sync.dma_start(out=outT[:, :], in_=ot[:, :])

    nc.compile()
    return nc


def _host_shards(label_mat, W1, a1, W2, a2, out_w, out_b, out2_w, out2_b, adj):
    f32 = np.float32
    bf16 = ml_dtypes.bfloat16
    label_T = np.asarray(label_mat, f32).T.astype(bf16)                 # [768, N]
    adjT = np.asarray(adj).T
    vnan = np.where(adjT == 1, np.float32(0.0), np.float32(np.nan)).astype(bf16)
    # elu is computed +1 shifted on device; fold the -1 back into the biases
    outb_adj = np.asarray(out_b, f32) - np.asarray(out_w, f32).sum(axis=1)
    out2b_adj = np.asarray(out2_b, f32) - np.asarray(out2_w, f32).sum(axis=1)
    common = dict(
        W1=np.ascontiguousarray(np.asarray(W1, f32).astype(bf16)),
        W1T=np.ascontiguousarray(np.asarray(W1, f32).transpose(0, 2, 1).astype(bf16)),
        a1p=np.ascontiguousarray(np.asarray(a1, f32).reshape(HEADS, 2, HID)
                                 .transpose(0, 2, 1).astype(bf16)),
        W2=np.ascontiguousarray(np.asarray(W2, f32).astype(bf16)),
        W2T=np.ascontiguousarray(np.asarray(W2, f32).transpose(0, 2, 1).astype(bf16)),
        a2p=np.ascontiguousarray(np.asarray(a2, f32).reshape(HEADS, 2, HID)
                                 .transpose(0, 2, 1).astype(bf16)),
        outwT=np.ascontiguousarray(np.asarray(out_w, f32).T.astype(bf16)),
        outb=np.ascontiguousarray(outb_adj.reshape(IN_DIM, 1)),
        out2wT=np.ascontiguousarray(np.asarray(out2_w, f32).T.astype(bf16)),
        out2b=np.ascontiguousarray(out2b_adj.reshape(OUT_DIM, 1)),
    )
    in_maps = []
    for c in range(NCORES):
        sl = slice(c * R, (c + 1) * R)
        m = dict(common)
        m["xT"] = np.ascontiguousarray(label_T[:, sl])
        m["vnan"] = np.ascontiguousarray(vnan[:, sl])
        in_maps.append(m)
    return in_maps


def kernel(**inputs):
    global _nc_cache, last_exec_time_ns
    if _nc_cache is None:
        _nc_cache = _build_program()
    nc = _nc_cache
    in_maps = _host_shards(**inputs)
    trace = os.environ.get("GAT_TRACE", "0") == "1"
    res = run_bass_kernel_spmd(nc, in_maps, list(range(NCORES)), trace=trace)
    last_exec_time_ns = res.exec_time_ns
    out = np.empty((N, OUT_DIM), np.float32)
    for c in range(NCORES):
        out[c * R:(c + 1) * R, :] = np.asarray(res.results[c]["outT"]).T
    return out


# revision 14
# speedup vs baseline: 1.0333x; 1.0333x over previous
"""Two-layer GAT (N=4096, 4 heads, HID=256) on 8 TRN2 NeuronCores.

Sharding: each core owns N/8 = 512 destination rows of every N^2 attention
matrix. Weights are replicated. Per head we compute the local projection
g_shard = h_shard @ W.T on the owning core, then AllGather a packed
[512, 260] payload (g | 1 | s_dst_hi | s_dst_lo | pad) so every core has
the full [4096, 260] projected features for the attention matmul.

Rank-1 softmax decomposition: because exp is monotone,
  exp(leakyrelu(u)) = max(e^u, e^{0.2u}),  u = s_src_i + s_dst_j
and on each branch e^u = e^{s_i} e^{s_j} is a rank-1 outer product. With
binary gates T1 = [u >= 0] * M and T2 = [u < 0] * M (M = adjacency):
  P = T1 o (a b^T) + T2 o (c d^T),   a=e^{s_src}, b=e^{s_dst},
                                     c=a^0.2,     d=b^0.2
so P @ g = a o (T1 @ (b o g)) + c o (T2 @ (d o g)): the N^2 exp/lrelu/mask
elementwise work of a direct softmax collapses to producing the two binary
matrices. T1/T2 come from one shared tensor_tensor add u' = sbc + Vnan
(Vnan = 0 where adjacent, NaN where not -- NaN fails both >= and <
comparisons, killing masked entries in both gates; sbc + 0 is exact in
bf16) plus two tensor_scalar compares. The i-side scalings fold into the
epilogue:  h' = (U1 + e o U2) / (D1 + e o D2),  e_i = e^{-0.8 s_src_i},
where D1/D2 ride along as column 256 of the matmul rhs (the payload's
ones column scaled by b resp. d).

Layout: attention tiles are [j=source (partition), i=dest (free)] so the
binary gates feed the PE matmul directly as lhsT. U1[ib]/U2[ib] are
[128, 257] accumulators, one PSUM bank each; a single shared 8-bank pool
works because phase A (all heads) fully precedes phase B (all heads) in
program order. s_dst crosses the bf16 payload as a hi+lo split to keep
full logit precision. All strided payload offsets are kept 4B-aligned so
the DVE compare/scale ops run in packed (2x/4x) modes.

elu is computed +1-shifted (elu(x)+1 = relu(x) + min(exp(x),1)) and the
-1 correction is folded into the next matmul's bias on the host
(outb -= out_w @ 1). Head outputs transpose via the DMA xbar, not the PE.

All matmul operands are bf16; accumulation is fp32 in PSUM.
"""

import os

import numpy as np
import ml_dtypes

import concourse.bass as bass
import concourse.tile as tile
from concourse import bacc, mybir
from concourse.bass_utils import run_bass_kernel_spmd

N, IN_DIM, HID, HEADS, OUT_DIM = 4096, 768, 256, 4, 32
ALPHA = 0.2
NCORES = 8
R = N // NCORES          # 512 rows per core
RB = R // 128            # 4 row blocks
FB = IN_DIM // 128       # 6 feature blocks
JB = N // 128            # 32 source chunks
HC = (HID * HEADS) // 128  # 8 concat-feature chunks
GW = HID + 4             # payload: g (256) | ones | s_hi | s_lo | pad
XW = 2 * HID + 8         # x12: [b*g | b | junk | pad] at 0, [d*g | d | junk | pad] at 260

F32 = mybir.dt.float32
BF16 = mybir.dt.bfloat16
AF = mybir.ActivationFunctionType
OP = mybir.AluOpType

last_exec_time_ns = None
_nc_cache = None

# engine for the u' = sbc + Vnan pass per source chunk: D = DVE, G = GPSIMD
U_ENG = list("DDGDDGDDGDDGDDGDDGDDGDDGDDGDDGDD")
assert len(U_ENG) == 32


def _build_layer(nc, tc, pools, x_tiles, W_ap, WT_ap, ap_ap, vnan_all, L):
    """One GAT layer. x_tiles: 6 SBUF tiles [128, R] bf16 (features x rows,
    feature-major). Returns 8 SBUF tiles [128, R] bf16 = concat-head
    activations transposed (x_gatT), elu+1 applied."""
    sb = pools["sb"]
    ps = pools["ps"]
    dram_pay = pools["dram_pay"]
    dram_gat = pools["dram_gat"]
    ones_row = pools["ones_row"]

    groups = [list(range(NCORES))]

    head_state = []
    # ---- Phase A: per-head projection + payload + AllGather ----
    for h in range(HEADS):
        W_t = []
        for cc in range(2):
            wt = sb.tile([128, IN_DIM], BF16, name=f"W_L{L}h{h}c{cc}", tag="Wh", bufs=4)
            nc.sync.dma_start(out=wt[:, :], in_=W_ap[h, cc * 128:(cc + 1) * 128, :])
            W_t.append(wt)
        WTaug = []
        for fb in range(FB):
            wta = sb.tile([128, HID + 2], BF16, name=f"WTa_L{L}h{h}f{fb}", tag="WTaug",
                          bufs=2 * FB)
            nc.sync.dma_start(out=wta[:, 0:HID], in_=WT_ap[h, fb * 128:(fb + 1) * 128, :])
            WTaug.append(wta)
        a_t = []
        for cc in range(2):
            at = sb.tile([128, 2], BF16, name=f"a_L{L}h{h}c{cc}", tag="ah", bufs=4)
            nc.sync.dma_start(out=at[:, :], in_=ap_ap[h, cc * 128:(cc + 1) * 128, :])
            a_t.append(at)

        # w_eff[f, 0:2] = W.T @ [a_src | a_dst]  -> [768, 2] in 6 blocks
        weff = []
        for fb in range(FB):
            pw = ps.tile([128, 512], F32, name=f"pw_L{L}h{h}f{fb}", tag="ps")
            for cc in range(2):
                nc.tensor.matmul(pw[:, 0:2], lhsT=W_t[cc][:, fb * 128:(fb + 1) * 128],
                                 rhs=a_t[cc][:, :], start=(cc == 0), stop=(cc == 1))
            wf = sb.tile([128, 2], BF16, name=f"weff_L{L}h{h}f{fb}", tag="weff",
                         bufs=2 * FB)
            nc.vector.tensor_copy(wf[:, :], pw[:, 0:2])
            # dst half -> column HID of the projection rhs (s_dst per row)
            nc.vector.tensor_copy(WTaug[fb][:, HID:HID + 1], wf[:, 1:2])
            # src half -> column HID+1 (s_src per row, for the epilogue scalars)
            nc.vector.tensor_copy(WTaug[fb][:, HID + 1:HID + 2], wf[:, 0:1])
            weff.append(wf)

        # s_srcT [1, R] = w_eff_src.T @ x  (free-axis layout for the broadcast)
        ps_s = ps.tile([128, R], F32, name=f"ps_s_L{L}h{h}", tag="ps")
        for fb in range(FB):
            nc.tensor.matmul(ps_s[0:1, :], lhsT=weff[fb][:, 0:1], rhs=x_tiles[fb][:, :],
                             start=(fb == 0), stop=(fb == FB - 1))
        ssrcT = sb.tile([1, R], BF16, name=f"ssrcT_L{L}h{h}", tag="ssrcT", bufs=2)
        nc.vector.tensor_copy(ssrcT[:, :], ps_s[0:1, :])

        # broadcast s_src across partitions: [128, R] bf16 (feeds the gate ops)
        pb = ps.tile([128, R], F32, name=f"pb_L{L}h{h}", tag="ps")
        nc.tensor.matmul(pb[:, :], lhsT=ones_row[0:1, :], rhs=ssrcT[:, :],
                         start=True, stop=True)
        sbc = sb.tile([128, R], BF16, name=f"sbc_L{L}h{h}", tag="sbc", bufs=HEADS)
        nc.vector.tensor_copy(sbc[:, :], pb[:, :])

        # g_aug = x.T @ WTaug -> per ib [128, 258] (g | s_dst | s_src), packed
        # into the payload as [g | 1 | s_hi | s_lo | pad]
        pay_t = dram_pay.tile([R, GW], BF16, name=f"pay_L{L}h{h}", tag="pay")
        pl = sb.tile([128, RB * GW], BF16, name=f"pl_L{L}h{h}", tag="pl", bufs=2)
        ecol = sb.tile([128, RB], F32, name=f"ecol_L{L}h{h}", tag="ecol", bufs=HEADS)
        for ib in range(RB):
            pg = ps.tile([128, 512], F32, name=f"pg_L{L}h{h}b{ib}", tag="ps")
            for fb in range(FB):
                nc.tensor.matmul(pg[:, 0:HID + 2],
                                 lhsT=x_tiles[fb][:, ib * 128:(ib + 1) * 128],
                                 rhs=WTaug[fb][:, :], start=(fb == 0),
                                 stop=(fb == FB - 1))
            o = ib * GW
            nc.vector.tensor_copy(pl[:, o:o + HID], pg[:, 0:HID])
            nc.vector.memset(pl[:, o + HID:o + HID + 1], 1.0)
            # s_dst split: hi = bf16(s), lo = bf16(s - hi)
            nc.vector.tensor_copy(pl[:, o + HID + 1:o + HID + 2], pg[:, HID:HID + 1])
            nc.vector.tensor_tensor(pl[:, o + HID + 2:o + HID + 3],
                                    pg[:, HID:HID + 1],
                                    pl[:, o + HID + 1:o + HID + 2], OP.subtract)
            # e_i = e^{-0.8 s_src_i}: per-partition epilogue scalar
            nc.scalar.activation(ecol[:, ib:ib + 1], pg[:, HID + 1:HID + 2], AF.Exp,
                                 scale=-0.8)
        # one DMA: SBUF [p, (ib, c)] -> DRAM [(ib, p), c]
        nc.sync.dma_start(out=pay_t.rearrange("(ib p) c -> p ib c", p=128),
                          in_=pl.rearrange("p (ib c) -> p ib c", c=GW))

        gat_t = dram_gat.tile([N, GW], BF16, name=f"gat_L{L}h{h}", tag="gat",
                              addr_space="Shared")
        nc.gpsimd.collective_compute(
            "AllGather", OP.bypass, replica_groups=groups,
            ins=[pay_t.opt()], outs=[gat_t.opt()],
        )
        head_state.append((gat_t, sbc, ecol))

    # ---- Phase B: attention per head ----
    xgatT = []
    for hc in range(HC):
        xg = sb.tile([128, R], BF16, name=f"xgatT_L{L}c{hc}", tag="xgatT", bufs=HC)
        xgatT.append(xg)

    for h in range(HEADS):
        gat_t, sbc, ecol = head_state[h]
        # one DMA: DRAM [(jj, p), c] -> SBUF [p, (jj, c)]
        gf = sb.tile([128, JB * GW], BF16, name=f"gf_L{L}h{h}", tag="gf", bufs=2)
        for q in range(4):
            js = slice(q * JB // 4, (q + 1) * JB // 4)
            nc.sync.dma_start(out=gf.rearrange("p (j c) -> p j c", c=GW)[:, js],
                              in_=gat_t.rearrange("(j p) c -> p j c", p=128)[:, js])
        gfv = gf.rearrange("p (j c) -> p j c", c=GW)
        # s_dst strips (hi + lo -> fp32), then b = e^s, d = e^{0.2 s}
        sh = sb.tile([128, JB], F32, name=f"sh_L{L}h{h}", tag="sh", bufs=2)
        sl = sb.tile([128, JB], F32, name=f"sl_L{L}h{h}", tag="sl", bufs=2)
        for q in range(4):
            js = slice(q * JB // 4, (q + 1) * JB // 4)
            nc.vector.tensor_copy(sh[:, js], gfv[:, js, HID + 1])
            nc.vector.tensor_copy(sl[:, js], gfv[:, js, HID + 2])
        sdst = sb.tile([128, JB], F32, name=f"sdst_L{L}h{h}", tag="sdst", bufs=2)
        nc.vector.tensor_tensor(sdst[:, :], sh[:, :], sl[:, :], OP.add)
        bd32 = sb.tile([128, 2 * JB], F32, name=f"bd32_L{L}h{h}", tag="bd32", bufs=2)
        bdv32 = bd32.rearrange("p (j two) -> p j two", two=2)
        nc.scalar.activation(bdv32[:, :, 0], sdst[:, :], AF.Exp)
        nc.scalar.activation(bdv32[:, :, 1], sdst[:, :], AF.Exp, scale=ALPHA)

        U1 = []
        U2 = []
        for ib in range(RB):
            U1.append(ps.tile([128, 512], F32, name=f"U1_L{L}h{h}b{ib}", tag="ps"))
            U2.append(ps.tile([128, 512], F32, name=f"U2_L{L}h{h}b{ib}", tag="ps"))

        for jj in range(JB):
            o = jj * GW
            # u' = sbc + Vnan (NaN where not adjacent -> both gates 0)
            up = sb.tile([128, R], BF16, name=f"up_L{L}h{h}j{jj}", tag="up", bufs=3)
            eng = nc.gpsimd if U_ENG[jj] == "G" else nc.vector
            eng.tensor_tensor(up[:, :], sbc[:, :], vnan_all[:, jj * R:(jj + 1) * R],
                              OP.add)
            # binary gates
            t1 = sb.tile([128, R], BF16, name=f"t1_L{L}h{h}j{jj}", tag="t1", bufs=3)
            nc.vector.tensor_scalar(t1[:, :], up[:, :], sdst[:, jj:jj + 1], 0.0,
                                    OP.add, OP.is_ge)
            t2 = sb.tile([128, R], BF16, name=f"t2_L{L}h{h}j{jj}", tag="t2", bufs=3)
            nc.vector.tensor_scalar(t2[:, :], up[:, :], sdst[:, jj:jj + 1], 0.0,
                                    OP.add, OP.is_lt)
            # scaled payload halves: [b*g | b | junk] and [d*g | d | junk]
            x12 = sb.tile([128, XW], BF16, name=f"x12_L{L}h{h}j{jj}", tag="x12",
                          bufs=3)
            nc.vector.tensor_scalar(x12[:, 0:HID + 2], gf[:, o:o + HID + 2],
                                    bd32[:, 2 * jj:2 * jj + 1], None, OP.mult)
            nc.scalar.activation(x12[:, HID + 4:2 * HID + 6], gf[:, o:o + HID + 2],
                                 AF.Copy, scale=bd32[:, 2 * jj + 1:2 * jj + 2])
            first, last = (jj == 0), (jj == JB - 1)
            for ib in range(RB):
                ibs = slice(ib * 128, (ib + 1) * 128)
                nc.tensor.matmul(U1[ib][:, 0:HID + 1], lhsT=t1[:, ibs],
                                 rhs=x12[:, 0:HID + 1], start=first, stop=last)
                nc.tensor.matmul(U2[ib][:, 0:HID + 1], lhsT=t2[:, ibs],
                                 rhs=x12[:, HID + 4:2 * HID + 5], start=first,
                                 stop=last)

        for ib in range(RB):
            # num = U1 + e o U2 (col 256 = D1 + e D2 = softmax denominator)
            tmp = sb.tile([128, HID + 1], F32, name=f"tmp_L{L}h{h}b{ib}", tag="tmp",
                          bufs=2)
            nc.scalar.activation(tmp[:, :], U2[ib][:, 0:HID + 1], AF.Copy,
                                 scale=ecol[:, ib:ib + 1])
            num = sb.tile([128, HID + 1], F32, name=f"num_L{L}h{h}b{ib}", tag="num",
                          bufs=2)
            nc.vector.tensor_tensor(num[:, :], U1[ib][:, 0:HID + 1], tmp[:, :], OP.add)
            rcp = sb.tile([128, 1], F32, name=f"rcp_L{L}h{h}b{ib}", tag="rcp", bufs=2)
            nc.vector.reciprocal(rcp[:, :], num[:, HID:HID + 1])
            hn = sb.tile([128, HID], BF16, name=f"hn_L{L}h{h}b{ib}", tag="hn", bufs=2)
            nc.vector.tensor_scalar(hn[:, :], num[:, 0:HID], rcp[:, 0:1], None,
                                    OP.mult)
            # elu+1 = relu(hn) + min(exp(hn), 1)
            r1 = sb.tile([128, HID], BF16, name=f"r1_L{L}h{h}b{ib}", tag="r1", bufs=2)
            nc.vector.tensor_scalar(r1[:, :], hn[:, :], 0.0, None, OP.max)
            e0 = sb.tile([128, HID], BF16, name=f"e0_L{L}h{h}b{ib}", tag="e0", bufs=2)
            nc.scalar.activation(e0[:, :], hn[:, :], AF.Exp)
            eo = sb.tile([128, HID], BF16, name=f"eo_L{L}h{h}b{ib}", tag="eo", bufs=2)
            nc.vector.scalar_tensor_tensor(eo[:, :], e0[:, :], 1.0, r1[:, :],
                                           OP.min, OP.add)
            for cb in range(2):
                nc.sync.dma_start_transpose(
                    out=xgatT[h * 2 + cb][:, ib * 128:(ib + 1) * 128],
                    in_=eo[:, cb * 128:(cb + 1) * 128])
    return xgatT


def _build_program():
    nc = bacc.Bacc("TRN2", target_bir_lowering=False, debug=False,
                   num_devices=NCORES)

    xT_in = nc.dram_tensor("xT", [IN_DIM, R], BF16, kind="ExternalInput").ap()
    vnan_in = nc.dram_tensor("vnan", [N, R], BF16, kind="ExternalInput").ap()
    W1_in = nc.dram_tensor("W1", [HEADS, HID, IN_DIM], BF16, kind="ExternalInput").ap()
    W1T_in = nc.dram_tensor("W1T", [HEADS, IN_DIM, HID], BF16, kind="ExternalInput").ap()
    a1_in = nc.dram_tensor("a1p", [HEADS, HID, 2], BF16, kind="ExternalInput").ap()
    W2_in = nc.dram_tensor("W2", [HEADS, HID, IN_DIM], BF16, kind="ExternalInput").ap()
    W2T_in = nc.dram_tensor("W2T", [HEADS, IN_DIM, HID], BF16, kind="ExternalInput").ap()
    a2_in = nc.dram_tensor("a2p", [HEADS, HID, 2], BF16, kind="ExternalInput").ap()
    outwT_in = nc.dram_tensor("outwT", [HID * HEADS, IN_DIM], BF16,
                              kind="ExternalInput").ap()
    outb_in = nc.dram_tensor("outb", [IN_DIM, 1], F32, kind="ExternalInput").ap()
    out2wT_in = nc.dram_tensor("out2wT", [HID * HEADS, OUT_DIM], BF16,
                               kind="ExternalInput").ap()
    out2b_in = nc.dram_tensor("out2b", [OUT_DIM, 1], F32, kind="ExternalInput").ap()
    outT = nc.dram_tensor("outT", [OUT_DIM, R], F32, kind="ExternalOutput").ap()

    with tile.TileContext(nc) as tc:
        with tc.tile_pool(name="sb", bufs=1) as sb, \
             tc.tile_pool(name="ps", bufs=8, space="PSUM") as ps, \
             tc.tile_pool(name="dram_pay", bufs=4, space="DRAM") as dram_pay, \
             tc.tile_pool(name="dram_gat", bufs=3, space="DRAM") as dram_gat:

            pools = dict(sb=sb, ps=ps, dram_pay=dram_pay, dram_gat=dram_gat)

            groups = [list(range(NCORES))]

            # warm up the collective path (absorbs the one-time barrier) with a
            # tiny gather issued before anything else on the gpsimd queue
            warm = sb.tile([1, 16], BF16, name="warm", tag="warm", bufs=1)
            nc.vector.memset(warm[:, :], 0.0)
            wpay = dram_pay.tile([1, 16], BF16, name="wpay", tag="wpay")
            nc.sync.dma_start(out=wpay[:, :], in_=warm[:, :])
            wgat = dram_gat.tile([NCORES, 16], BF16, name="wgat", tag="wgat",
                                 addr_space="Shared")
            nc.gpsimd.collective_compute(
                "AllGather", OP.bypass, replica_groups=groups,
                ins=[wpay.opt()], outs=[wgat.opt()],
            )

            ones_row = sb.tile([1, 128], BF16, name="ones_row", tag="ones_row", bufs=1)
            nc.vector.memset(ones_row[:, :], 1.0)
            pools["ones_row"] = ones_row

            # resident inputs
            x0 = []
            for fb in range(FB):
                x = sb.tile([128, R], BF16, name=f"x0_{fb}", tag="x0", bufs=FB)
                nc.sync.dma_start(out=x[:, :], in_=xT_in[fb * 128:(fb + 1) * 128, :])
                x0.append(x)
            outw_t = []
            for hc in range(HC):
                w = sb.tile([128, IN_DIM], BF16, name=f"outw{hc}", tag="outw", bufs=HC)
                nc.sync.dma_start(out=w[:, :], in_=outwT_in[hc * 128:(hc + 1) * 128, :])
                outw_t.append(w)
            out2w_t = []
            for hc in range(HC):
                w = sb.tile([128, OUT_DIM], BF16, name=f"out2w{hc}", tag="out2w",
                            bufs=HC)
                nc.sync.dma_start(out=w[:, :], in_=out2wT_in[hc * 128:(hc + 1) * 128, :])
                out2w_t.append(w)
            outb_t = []
            for fb in range(FB):
                b = sb.tile([128, 1], F32, name=f"outb{fb}", tag="outb", bufs=FB)
                nc.sync.dma_start(out=b[:, :], in_=outb_in[fb * 128:(fb + 1) * 128, :])
                outb_t.append(b)
            out2b_t = sb.tile([OUT_DIM, 1], F32, name="out2b", tag="out2b", bufs=1)
            nc.sync.dma_start(out=out2b_t[:, :], in_=out2b_in[:, :])

            # adjacency gate-killer: 0 where adjacent, NaN where not
            vnan_all = sb.tile([128, JB * R], BF16, name="vnan_all", tag="vnan",
                               bufs=1)
            for q in range(4):
                js = slice(q * JB // 4, (q + 1) * JB // 4)
                nc.sync.dma_start(
                    out=vnan_all.rearrange("p (j c) -> p j c", c=R)[:, js],
                    in_=vnan_in.rearrange("(j p) c -> p j c", p=128)[:, js])

            # ---- layer 1 ----
            xg1 = _build_layer(nc, tc, pools, x0, W1_in, W1T_in, a1_in, vnan_all, 1)
            x1 = []
            for fb in range(FB):
                px = ps.tile([128, R], F32, name=f"px1_{fb}", tag="ps")
                for hc in range(HC):
                    nc.tensor.matmul(px[:, :], lhsT=outw_t[hc][:, fb * 128:(fb + 1) * 128],
                                     rhs=xg1[hc][:, :], start=(hc == 0),
                                     stop=(hc == HC - 1))
                x = sb.tile([128, R], BF16, name=f"x1_{fb}", tag="x1", bufs=FB)
                nc.vector.tensor_scalar(x[:, :], px[:, :], outb_t[fb][:, 0:1], None,
                                        OP.add)
                x1.append(x)

            # ---- layer 2 ----
            xg2 = _build_layer(nc, tc, pools, x1, W2_in, W2T_in, a2_in, vnan_all, 2)
            po = ps.tile([128, R], F32, name="po", tag="ps")
            for hc in range(HC):
                nc.tensor.matmul(po[0:OUT_DIM, :], lhsT=out2w_t[hc][:, 0:OUT_DIM],
                                 rhs=xg2[hc][:, :], start=(hc == 0),
                                 stop=(hc == HC - 1))
            ot = sb.tile([OUT_DIM, R], F32, name="ot", tag="ot", bufs=1)
            nc.vector.tensor_scalar(ot[:, :], po[0:OUT_DIM, :], out2b_t[:, 0:1], None,
                                    OP.add)
            nc.sync.dma_start(out=outT[:, :], in_=ot[:, :])

    nc.compile()
    return nc


def _host_shards(label_mat, W1, a1, W2, a2, out_w, out_b, out2_w, out2_b, adj):
    f32 = np.float32
    bf16 = ml_dtypes.bfloat16
    label_T = np.asarray(label_mat, f32).T.astype(bf16)                 # [768, N]
    adjT = np.asarray(adj).T
    vnan = np.where(adjT == 1, np.float32(0.0), np.float32(np.nan)).astype(bf16)
    # elu is computed +1 shifted on device; fold the -1 back into the biases
    outb_adj = np.asarray(out_b, f32) - np.asarray(out_w, f32).sum(axis=1)
    out2b_adj = np.asarray(out2_b, f32) - np.asarray(out2_w, f32).sum(axis=1)
    common = dict(
        W1=np.ascontiguousarray(np.asarray(W1, f32).astype(bf16)),
        W1T=np.ascontiguousarray(np.asarray(W1, f32).transpose(0, 2, 1).astype(bf16)),
        a1p=np.ascontiguousarray(np.asarray(a1, f32).reshape(HEADS, 2, HID)
                                 .transpose(0, 2, 1).astype(bf16)),
        W2=np.ascontiguousarray(np.asarray(W2, f32).astype(bf16)),
        W2T=np.ascontiguousarray(np.asarray(W2, f32).transpose(0, 2, 1).astype(bf16)),
        a2p=np.ascontiguousarray(np.asarray(a2, f32).reshape(HEADS, 2, HID)
                                 .transpose(0, 2, 1).astype(bf16)),
        outwT=np.ascontiguousarray(np.asarray(out_w, f32).T.astype(bf16)),
        outb=np.ascontiguousarray(outb_adj.reshape(IN_DIM, 1)),
        out2wT=np.ascontiguousarray(np.asarray(out2_w, f32).T.astype(bf16)),
        out2b=np.ascontiguousarray(out2b_adj.reshape(OUT_DIM, 1)),
    )
    in_maps = []
    for c in range(NCORES):
        sl = slice(c * R, (c + 1) * R)
        m = dict(common)
        m["xT"] = np.ascontiguousarray(label_T[:, sl])
        m["vnan"] = np.ascontiguousarray(vnan[:, sl])
        in_maps.append(m)
    return in_maps


def kernel(**inputs):
    global _nc_cache, last_exec_time_ns
    if _nc_cache is None:
        _nc_cache = _build_program()
    nc = _nc_cache
    in_maps = _host_shards(**inputs)
    trace = os.environ.get("GAT_TRACE", "0") == "1"
    res = run_bass_kernel_spmd(nc, in_maps, list(range(NCORES)), trace=trace)
    last_exec_time_ns = res.exec_time_ns
    out = np.empty((N, OUT_DIM), np.float32)
    for c in range(NCORES):
        out[c * R:(c + 1) * R, :] = np.asarray(res.results[c]["outT"]).T
    return out
